# revision 16
# baseline (speedup 1.0000x reference)
"""MixProp GNN message passing on 8 Trainium2 NeuronCores.

Reference computation (per batch element b):
    h0 = x;  h_k = alpha*x + (1-alpha) * (adj @ h_{k-1})   k=1..3   (matmul over nodes)
    ho = concat([h0..h3], channel axis);  out = W @ ho + b          (1x1 conv)

Node-propagation (node axis) commutes with channel mixing (channel
axis), so the alpha-blending folds into the conv weights on the host:
    out = sum_k M_k @ (A^k x) + b
with M_0 = W0 + a(W1+W2+W3), M_1 = B(W1 + aW2 + aW3),
     M_2 = B^2(W2 + aW3),    M_3 = B^3 W3,   (a=alpha, B=1-alpha)

Sharding: data-parallel over batch B=8, one batch element per core;
adj (host-pre-transposed) and conv weights replicated.

Device dataflow per core (fp16 operands, fp32 PSUM accumulation),
pipelined over T in chunks of 16 time steps:
  YK [128 w-part, 4 node tiles, 16 t, 128 kc]   kc = k*32 + c
    slot k=0 <- x (ACT copy from the contiguous chunk load)
    slot k   <- prop step k psum (PE contracts nodes; DVE/ACT evac)
  per node tile: XBAR DMA-transpose [v, (t,kc)] -> Z [kc, t, v] on-chip
    (no HBM scratch round trip)
  conv: one K=128 matmul per (node tile, t): psum[128 v, 32 o] with
    ap_size=32 (4x fewer PE rows than the [o, (v,t)] orientation)
  psum -> stage [v, o, t] f32 (DVE add folds the bias, (t,o)->(o,t))
  out DMA per node tile in two slabs: t 0:128 (512B descriptors,
  overlapped with the tail chunks) + t 128:168.
"""

import sys

import numpy as np

sys.path.insert(0, "/opt/trn_rl_repo")

from contextlib import ExitStack

GDEP = 3
ALPHA = 0.05
Y3_SCALE = 1.0 / 128.0   # keep |y3| inside fp16 range; folded into M3
C = 32            # channels
N = 512           # nodes
T = 168           # time steps
B = 8             # batch == n_cores
P = 128           # partitions
NVT = N // P      # 4 node tiles
KC = (GDEP + 1) * C   # 128 stacked (k, c) rows for the conv

TC = 16                                      # t-chunk size
CHUNKS = [(i * TC, TC) for i in range(9)] + [(144, 12), (156, 8), (164, 4)]

_NC_CACHE = {}


def _build_nc():
    import concourse.mybir as mybir
    import concourse.tile as tile
    from concourse import bacc

    f32 = mybir.dt.float32
    f16 = mybir.dt.float16

    nc = bacc.Bacc("TRN2", target_bir_lowering=False, debug=False, num_devices=B)

    xprop = nc.dram_tensor("xprop", [P, NVT, T, C], f16, kind="ExternalInput").ap()
    adjT16 = nc.dram_tensor("adjT16", [N, N], f16, kind="ExternalInput").ap()
    mt16 = nc.dram_tensor("mt16", [KC, C], f16, kind="ExternalInput").ap()
    bias512 = nc.dram_tensor("bias512", [P, 512], f32, kind="ExternalInput").ap()
    out = nc.dram_tensor("out", [NVT, P, T, C], f32, kind="ExternalOutput").ap()

    with tile.TileContext(nc) as tc, ExitStack() as ctx:
        _emit(ctx, tc, nc, mybir, xprop, adjT16, mt16, bias512, out)

    nc.compile()
    return nc


def _emit(ctx, tc, nc, mybir, xprop, adjT16, mt16, bias512, out):
    f32 = mybir.dt.float32
    f16 = mybir.dt.float16
    Identity = mybir.ActivationFunctionType.Identity

    const_pool = ctx.enter_context(tc.tile_pool(name="const", bufs=1))
    xp_pool = ctx.enter_context(tc.tile_pool(name="xp", bufs=5))
    yk_pool = ctx.enter_context(tc.tile_pool(name="yk", bufs=5))
    z_pool = ctx.enter_context(tc.tile_pool(name="z", bufs=12))
    s_pool = ctx.enter_context(tc.tile_pool(name="s", bufs=6))
    psum_pool = ctx.enter_context(tc.tile_pool(name="psum", bufs=4, space="PSUM"))
    cpsum_pool = ctx.enter_context(tc.tile_pool(name="cpsum", bufs=4, space="PSUM"))

    # ---- startup: interleave adj / first x chunk per node tile so the
    # first step-1 matmul's dependencies land as early as possible ----
    adj_sb = const_pool.tile([P, NVT, N], f16, tag="adj")
    adj_v = adjT16.rearrange("(wt wp) v -> wp wt v", wp=P)
    xp0 = xp_pool.tile([P, NVT, TC, C], f16, tag="xp")
    t00, tn0 = CHUNKS[0]
    nc.sync.dma_start(adj_sb[:, 0], adj_v[:, 0])
    nc.sync.dma_start(xp0[:, 0, :tn0, :], xprop[:, 0, t00:t00 + tn0, :])
    for wt in range(1, NVT):
        nc.sync.dma_start(adj_sb[:, wt], adj_v[:, wt])
        nc.sync.dma_start(xp0[:, wt, :tn0, :], xprop[:, wt, t00:t00 + tn0, :])

    mt_sb = const_pool.tile([KC, C], f16, tag="mt")
    bias_sb = const_pool.tile([P, 512], f32, tag="bias")
    nc.sync.dma_start(mt_sb[:], mt16)
    nc.sync.dma_start(bias_sb[:], bias512)
    xp1 = xp_pool.tile([P, NVT, TC, C], f16, tag="xp")
    t01, tn1 = CHUNKS[1]
    nc.sync.dma_start(xp1[:, :, :tn1, :], xprop[:, :, t01:t01 + tn1, :])
    xp2 = xp_pool.tile([P, NVT, TC, C], f16, tag="xp")
    t02, tn2 = CHUNKS[2]
    nc.sync.dma_start(xp2[:, :, :tn2, :], xprop[:, :, t02:t02 + tn2, :])

    def emit_steps(ci, t0, tn, xp, after_step):
        """3 propagation steps + per-node-tile XBAR transpose; invokes
        after_step(s) between steps to fill PE stall slots with the
        previous chunk's conv."""
        yk = yk_pool.tile([P, NVT, TC, KC], f16, tag="yk")
        zs = []
        for k in (1, 2, 3):
            for vt in range(NVT):
                ps = psum_pool.tile([P, 512], f32, tag="ps")
                for wt in range(NVT):
                    rhs = (xp[:, wt, :tn, :] if k == 1
                           else yk[:, wt, :tn, C * (k - 1):C * k])
                    nc.tensor.matmul(
                        ps[:, :tn * C],
                        adj_sb[:, wt, vt * P:(vt + 1) * P],
                        rhs,
                        start=(wt == 0),
                        stop=(wt == NVT - 1),
                    )
                src = ps[:, :tn * C].rearrange("p (t c) -> p t c", c=C)
                dst = yk[:, vt, :tn, C * k:C * (k + 1)]
                if k == 3:
                    # scale guards fp16 range; ACT while DVE drains 1/2
                    nc.scalar.activation(dst, src, Identity, scale=Y3_SCALE)
                    # x -> stacked slot k=0, per node tile, just before
                    # its XBAR (keeps the ACT queue free of long waits)
                    nc.scalar.activation(
                        yk[:, vt, :tn, 0:C], xp[:, vt, :tn, :], Identity
                    )
                    z = z_pool.tile([P, TC, P], f16, tag="z")
                    nc.sync.dma_start(
                        z[:, :tn, :], yk[:, vt, :tn, :], transpose=True
                    )
                    zs.append(z)
                elif vt < 2:
                    nc.vector.tensor_copy(dst, src)
                else:
                    # split step evacs DVE/ACT: halves the queue that
                    # paces each step transition
                    nc.scalar.activation(dst, src, Identity)
            after_step(k)
        return (t0, tn, zs)

    def emit_conv_vt(state, vt):
        """Conv for one node tile of a finished chunk: tn ap32 matmuls
        into one psum bank, bias-add evac (contiguous (t,o)), flush."""
        t0, tn, zs = state
        z = zs[vt]
        cps = cpsum_pool.tile([P, 512], f32, tag="cps")
        for i in range(tn):
            nc.tensor.matmul(
                cps[:, C * i:C * (i + 1)],
                z[:, i, :],
                mt_sb[:],
                start=True,
                stop=True,
            )
        s = s_pool.tile([P, 512], f32, tag="s")
        nc.vector.tensor_add(s[:, :tn * C], cps[:, :tn * C], bias_sb[:, :tn * C])
        nc.sync.dma_start(
            out[vt, :, t0:t0 + tn, :],
            s[:, :tn * C].rearrange("p (t o) -> p t o", o=C),
        )

    pend = []                      # chunks whose conv is not yet emitted
    xps = {0: xp0, 1: xp1, 2: xp2}
    for ci, (t0, tn) in enumerate(CHUNKS):
        # prefetch x two chunks ahead: the SP queue head-of-line
        # blocks on the previous chunk's last XBAR semaphore, so a
        # distance-1 prefetch only lands at the chunk boundary; at
        # distance 2 the load is in flight a full chunk before use
        if ci + 3 < len(CHUNKS):
            nt0, ntn = CHUNKS[ci + 3]
            nxp = xp_pool.tile([P, NVT, TC, C], f16, tag="xp")
            nc.sync.dma_start(nxp[:, :, :ntn, :], xprop[:, :, nt0:nt0 + ntn, :])
            xps[ci + 3] = nxp
        xp = xps.pop(ci)

        # conv lagged two chunks behind the propagation: the
        # evac3 -> XBAR -> conv semaphore chain is ~7us; a full chunk
        # of step matmuls (~11us) must sit between step3(c) and
        # conv(c) or the PE stalls (and drops out of max p-state)
        prev = pend.pop(0) if len(pend) >= 2 else None

        def after_step(k, prev=prev):
            if prev is None:
                return
            if k == 1:
                emit_conv_vt(prev, 0)
                emit_conv_vt(prev, 1)
            elif k == 2:
                emit_conv_vt(prev, 2)
                emit_conv_vt(prev, 3)

        pend.append(emit_steps(ci, t0, tn, xp, after_step))
    for state in pend:
        for vt in range(NVT):
            emit_conv_vt(state, vt)


def _get_nc():
    if "nc" not in _NC_CACHE:
        _NC_CACHE["nc"] = _build_nc()
    return _NC_CACHE["nc"]


def _host_prep(adj, W, b):
    """Host-side constant folding: transposed adj, mixed conv weights."""
    a, beta = ALPHA, 1.0 - ALPHA
    W = np.asarray(W, dtype=np.float32)
    W0, W1, W2, W3 = (W[:, i * C:(i + 1) * C] for i in range(4))
    M0 = W0 + a * (W1 + W2 + W3)
    M1 = beta * (W1 + a * W2 + a * W3)
    M2 = beta * beta * (W2 + a * W3)
    M3 = beta * beta * beta * W3 / Y3_SCALE
    mt16 = np.ascontiguousarray(
        np.concatenate([M0.T, M1.T, M2.T, M3.T], axis=0)
    ).astype(np.float16)  # [128, 32]: row (k*32+c), col o = M_k[o, c]
    bias512 = np.ascontiguousarray(
        np.tile(np.asarray(b, dtype=np.float32)[None, :], (P, TC))
    )  # [128, 512]: col (t'*32+o) = b[o]
    adjT16 = np.ascontiguousarray(np.asarray(adj, dtype=np.float32).T).astype(
        np.float16
    )
    return adjT16, mt16, bias512


def make_in_maps(x, adj, W, b):
    adjT16, mt16, bias512 = _host_prep(adj, W, b)
    x16 = np.asarray(x, dtype=np.float32).astype(np.float16)
    # [B, C, N, T] -> [B, 128 wp, 4 wt, T, C]
    xprop = np.ascontiguousarray(
        x16.reshape(B, C, NVT, P, T).transpose(0, 3, 2, 4, 1)
    )
    return [
        {
            "xprop": xprop[i],
            "adjT16": adjT16,
            "mt16": mt16,
            "bias512": bias512,
        }
        for i in range(B)
    ]


def _get_runner():
    """Reusable jitted SPMD executor (safe to invoke repeatedly, unlike
    per-call run_bass_kernel_spmd under axon)."""
    if "runner" in _NC_CACHE:
        return _NC_CACHE["runner"]
    import jax
    from jax.sharding import Mesh, PartitionSpec
    try:
        from jax import shard_map
    except ImportError:
        from jax.experimental.shard_map import shard_map
    from concourse import bass2jax, mybir

    nc = _get_nc()
    bass2jax.install_neuronx_cc_hook()

    pname = nc.partition_id_tensor.name if nc.partition_id_tensor else None
    in_names, out_names, out_avals, zero_outs = [], [], [], []
    for alloc in nc.m.functions[0].allocations:
        if not isinstance(alloc, mybir.MemoryLocationSet):
            continue
        name = alloc.memorylocations[0].name
        if alloc.kind == "ExternalInput":
            if name != pname:
                in_names.append(name)
        elif alloc.kind == "ExternalOutput":
            out_names.append(name)
            shape = tuple(alloc.tensor_shape)
            dtype = mybir.dt.np(alloc.dtype)
            out_avals.append(jax.core.ShapedArray(shape, dtype))
            zero_outs.append(np.zeros(shape, dtype))
    n_params = len(in_names)
    in_names_all = list(in_names) + out_names
    if pname is not None:
        in_names_all.append(pname)

    def _body(*args):
        operands = list(args)
        if pname is not None:
            operands.append(bass2jax.partition_id_tensor())
        return tuple(
            bass2jax._bass_exec_p.bind(
                *operands,
                out_avals=tuple(out_avals),
                in_names=tuple(in_names_all),
                out_names=tuple(out_names),
                lowering_input_output_aliases=(),
                sim_require_finite=True,
                sim_require_nnan=True,
                nc=nc,
            )
        )

    devices = jax.devices()[:B]
    mesh = Mesh(np.asarray(devices), ("core",))
    fn = jax.jit(
        shard_map(
            _body,
            mesh=mesh,
            in_specs=(PartitionSpec("core"),) * (n_params + len(out_names)),
            out_specs=(PartitionSpec("core"),) * len(out_names),
            check_rep=False,
        ),
        keep_unused=True,
    )

    def run(in_maps):
        per_core = [[np.asarray(m[nm]) for nm in in_names] for m in in_maps]
        concat_in = [
            np.concatenate([per_core[c][i] for c in range(B)], axis=0)
            for i in range(n_params)
        ]
        concat_zero = [np.concatenate([z] * B, axis=0) for z in zero_outs]
        outs = fn(*concat_in, *concat_zero)
        oi = out_names.index("out")
        full = np.asarray(outs[oi])
        per_core_rows = out_avals[oi].shape[0]
        return full.reshape(B, per_core_rows, *out_avals[oi].shape[1:])

    _NC_CACHE["runner"] = run
    return run


def _host_untranspose(outD):
    # device writes [NVT, P v, T, C o]; reference layout is [C, N, T]
    return np.ascontiguousarray(
        outD.reshape(B, N, T, C).transpose(0, 3, 1, 2)
    )


def kernel(x, adj, W, b):
    in_maps = make_in_maps(x, adj, W, b)
    try:
        run = _get_runner()
        return _host_untranspose(run(in_maps))
    except Exception:
        from concourse.bass_utils import run_bass_kernel_spmd

        res = run_bass_kernel_spmd(_get_nc(), in_maps, list(range(B)))
        outD = np.stack([res.results[i]["out"] for i in range(B)], axis=0)
        return _host_untranspose(outD)


# revision 17
# speedup vs baseline: 1.0450x; 1.0450x over previous
"""MixProp GNN message passing on 8 Trainium2 NeuronCores.

Reference computation (per batch element b):
    h0 = x;  h_k = alpha*x + (1-alpha) * (adj @ h_{k-1})   k=1..3   (matmul over nodes)
    ho = concat([h0..h3], channel axis);  out = W @ ho + b          (1x1 conv)

Node-propagation (node axis) commutes with channel mixing (channel
axis), so the alpha-blending folds into the conv weights on the host:
    out = sum_k M_k @ (A^k x) + b
with M_0 = W0 + a(W1+W2+W3), M_1 = B(W1 + aW2 + aW3),
     M_2 = B^2(W2 + aW3),    M_3 = B^3 W3,   (a=alpha, B=1-alpha)

Sharding: data-parallel over batch B=8, one batch element per core;
adj (host-pre-transposed) and conv weights replicated.

Device dataflow per core (fp16 operands, fp32 PSUM accumulation),
pipelined over T in chunks of 16 time steps:
  YK [128 w-part, 4 node tiles, 16 t, 128 kc]   kc = k*32 + c
    slot k=0 <- x (ACT copy from the contiguous chunk load)
    slot k   <- prop step k psum (PE contracts nodes; DVE/ACT evac)
  per node tile: XBAR DMA-transpose [v, (t,kc)] -> Z [kc, t, v] on-chip
    (no HBM scratch round trip)
  conv: one K=128 matmul per (node tile, t): psum[128 v, 32 o] with
    ap_size=32 (4x fewer PE rows than the [o, (v,t)] orientation)
  psum -> stage [v, o, t] f32 (DVE add folds the bias, (t,o)->(o,t))
  out DMA per node tile in two slabs: t 0:128 (512B descriptors,
  overlapped with the tail chunks) + t 128:168.
"""

import sys

import numpy as np

sys.path.insert(0, "/opt/trn_rl_repo")

from contextlib import ExitStack

GDEP = 3
ALPHA = 0.05
Y3_SCALE = 1.0 / 128.0   # keep |y3| inside fp16 range; folded into M3
C = 32            # channels
N = 512           # nodes
T = 168           # time steps
B = 8             # batch == n_cores
P = 128           # partitions
NVT = N // P      # 4 node tiles
KC = (GDEP + 1) * C   # 128 stacked (k, c) rows for the conv

TC = 16                                      # t-chunk size
CHUNKS = [(i * TC, TC) for i in range(10)] + [(160, 8)]

_NC_CACHE = {}


def _build_nc():
    import concourse.mybir as mybir
    import concourse.tile as tile
    from concourse import bacc

    f32 = mybir.dt.float32
    f16 = mybir.dt.float16

    nc = bacc.Bacc("TRN2", target_bir_lowering=False, debug=False, num_devices=B)

    xprop = nc.dram_tensor("xprop", [P, NVT, T, C], f16, kind="ExternalInput").ap()
    adjT16 = nc.dram_tensor("adjT16", [N, N], f16, kind="ExternalInput").ap()
    mt16 = nc.dram_tensor("mt16", [KC, C], f16, kind="ExternalInput").ap()
    bias512 = nc.dram_tensor("bias512", [P, 512], f32, kind="ExternalInput").ap()
    out = nc.dram_tensor("out", [NVT, P, T, C], f32, kind="ExternalOutput").ap()

    with tile.TileContext(nc) as tc, ExitStack() as ctx:
        _emit(ctx, tc, nc, mybir, xprop, adjT16, mt16, bias512, out)

    nc.compile()
    return nc


def _emit(ctx, tc, nc, mybir, xprop, adjT16, mt16, bias512, out):
    f32 = mybir.dt.float32
    f16 = mybir.dt.float16
    Identity = mybir.ActivationFunctionType.Identity

    const_pool = ctx.enter_context(tc.tile_pool(name="const", bufs=1))
    xp_pool = ctx.enter_context(tc.tile_pool(name="xp", bufs=5))
    yk_pool = ctx.enter_context(tc.tile_pool(name="yk", bufs=5))
    z_pool = ctx.enter_context(tc.tile_pool(name="z", bufs=12))
    s_pool = ctx.enter_context(tc.tile_pool(name="s", bufs=6))
    psum_pool = ctx.enter_context(tc.tile_pool(name="psum", bufs=4, space="PSUM"))
    cpsum_pool = ctx.enter_context(tc.tile_pool(name="cpsum", bufs=4, space="PSUM"))

    # ---- startup: interleave adj / first x chunk per node tile so the
    # first step-1 matmul's dependencies land as early as possible ----
    adj_sb = const_pool.tile([P, NVT, N], f16, tag="adj")
    adj_v = adjT16.rearrange("(wt wp) v -> wp wt v", wp=P)
    xp0 = xp_pool.tile([P, NVT, TC, C], f16, tag="xp")
    t00, tn0 = CHUNKS[0]
    nc.sync.dma_start(adj_sb[:, 0], adj_v[:, 0])
    nc.sync.dma_start(xp0[:, 0, :tn0, :], xprop[:, 0, t00:t00 + tn0, :])
    for wt in range(1, NVT):
        nc.sync.dma_start(adj_sb[:, wt], adj_v[:, wt])
        nc.sync.dma_start(xp0[:, wt, :tn0, :], xprop[:, wt, t00:t00 + tn0, :])

    mt_sb = const_pool.tile([KC, C], f16, tag="mt")
    bias_sb = const_pool.tile([P, 512], f32, tag="bias")
    nc.sync.dma_start(mt_sb[:], mt16)
    nc.sync.dma_start(bias_sb[:], bias512)
    xp1 = xp_pool.tile([P, NVT, TC, C], f16, tag="xp")
    t01, tn1 = CHUNKS[1]
    nc.sync.dma_start(xp1[:, :, :tn1, :], xprop[:, :, t01:t01 + tn1, :])
    xp2 = xp_pool.tile([P, NVT, TC, C], f16, tag="xp")
    t02, tn2 = CHUNKS[2]
    nc.sync.dma_start(xp2[:, :, :tn2, :], xprop[:, :, t02:t02 + tn2, :])

    def emit_steps(ci, t0, tn, xp, after_step):
        """3 propagation steps + per-node-tile XBAR transpose; invokes
        after_step(s) between steps to fill PE stall slots with the
        previous chunk's conv."""
        yk = yk_pool.tile([P, NVT, TC, KC], f16, tag="yk")
        zs = []
        for k in (1, 2, 3):
            for vt in range(NVT):
                for ta in range(0, tn, 16):
                    tb = min(ta + 16, tn)
                    ps = psum_pool.tile([P, 512], f32, tag="ps")
                    for wt in range(NVT):
                        rhs = (xp[:, wt, ta:tb, :] if k == 1
                               else yk[:, wt, ta:tb, C * (k - 1):C * k])
                        nc.tensor.matmul(
                            ps[:, :(tb - ta) * C],
                            adj_sb[:, wt, vt * P:(vt + 1) * P],
                            rhs,
                            start=(wt == 0),
                            stop=(wt == NVT - 1),
                        )
                    src = ps[:, :(tb - ta) * C].rearrange(
                        "p (t c) -> p t c", c=C
                    )
                    dst = yk[:, vt, ta:tb, C * k:C * (k + 1)]
                    if k == 3 or (vt < 2):
                        nc.vector.tensor_copy(dst, src) if k < 3 else                             nc.scalar.activation(dst, src, Identity,
                                                 scale=Y3_SCALE)
                    else:
                        nc.scalar.activation(dst, src, Identity)
                if k == 3:
                    # x -> stacked slot k=0, per node tile, just before
                    # its XBAR (keeps the ACT queue free of long waits)
                    nc.scalar.activation(
                        yk[:, vt, :tn, 0:C], xp[:, vt, :tn, :], Identity
                    )
                    z = z_pool.tile([P, TC, P], f16, tag="z")
                    nc.sync.dma_start(
                        z[:, :tn, :], yk[:, vt, :tn, :], transpose=True
                    )
                    zs.append(z)
            after_step(k)
        return (t0, tn, zs)

    def emit_conv_vt(state, vt):
        """Conv for one node tile of a finished chunk: tn ap32 matmuls
        into one psum bank, bias-add evac (contiguous (t,o)), flush."""
        t0, tn, zs = state
        z = zs[vt]
        s = s_pool.tile([P, TC * C], f32, tag="s")
        for ta in range(0, tn, 16):
            tb = min(ta + 16, tn)
            cps = cpsum_pool.tile([P, 512], f32, tag="cps")
            for i in range(ta, tb):
                nc.tensor.matmul(
                    cps[:, C * (i - ta):C * (i - ta + 1)],
                    z[:, i, :],
                    mt_sb[:],
                    start=True,
                    stop=True,
                )
            nc.vector.tensor_add(
                s[:, ta * C:tb * C],
                cps[:, :(tb - ta) * C],
                bias_sb[:, :(tb - ta) * C],
            )
        nc.sync.dma_start(
            out[vt, :, t0:t0 + tn, :],
            s[:, :tn * C].rearrange("p (t o) -> p t o", o=C),
        )

    pend = []                      # chunks whose conv is not yet emitted
    xps = {0: xp0, 1: xp1, 2: xp2}
    for ci, (t0, tn) in enumerate(CHUNKS):
        # prefetch x two chunks ahead: the SP queue head-of-line
        # blocks on the previous chunk's last XBAR semaphore, so a
        # distance-1 prefetch only lands at the chunk boundary; at
        # distance 2 the load is in flight a full chunk before use
        if ci + 3 < len(CHUNKS):
            nt0, ntn = CHUNKS[ci + 3]
            nxp = xp_pool.tile([P, NVT, TC, C], f16, tag="xp")
            nc.sync.dma_start(nxp[:, :, :ntn, :], xprop[:, :, nt0:nt0 + ntn, :])
            xps[ci + 3] = nxp
        xp = xps.pop(ci)

        # conv lagged two chunks behind the propagation: the
        # evac3 -> XBAR -> conv semaphore chain is ~7us; a full chunk
        # of step matmuls (~11us) must sit between step3(c) and
        # conv(c) or the PE stalls (and drops out of max p-state)
        prev = pend.pop(0) if len(pend) >= 2 else None

        def after_step(k, prev=prev):
            if prev is None:
                return
            if k == 1:
                emit_conv_vt(prev, 0)
                emit_conv_vt(prev, 1)
            elif k == 2:
                emit_conv_vt(prev, 2)
                emit_conv_vt(prev, 3)

        pend.append(emit_steps(ci, t0, tn, xp, after_step))
    for state in pend:
        for vt in range(NVT):
            emit_conv_vt(state, vt)


def _get_nc():
    if "nc" not in _NC_CACHE:
        _NC_CACHE["nc"] = _build_nc()
    return _NC_CACHE["nc"]


def _host_prep(adj, W, b):
    """Host-side constant folding: transposed adj, mixed conv weights."""
    a, beta = ALPHA, 1.0 - ALPHA
    W = np.asarray(W, dtype=np.float32)
    W0, W1, W2, W3 = (W[:, i * C:(i + 1) * C] for i in range(4))
    M0 = W0 + a * (W1 + W2 + W3)
    M1 = beta * (W1 + a * W2 + a * W3)
    M2 = beta * beta * (W2 + a * W3)
    M3 = beta * beta * beta * W3 / Y3_SCALE
    mt16 = np.ascontiguousarray(
        np.concatenate([M0.T, M1.T, M2.T, M3.T], axis=0)
    ).astype(np.float16)  # [128, 32]: row (k*32+c), col o = M_k[o, c]
    bias512 = np.ascontiguousarray(
        np.tile(np.asarray(b, dtype=np.float32)[None, :], (P, TC))
    )  # [128, 512]: col (t'*32+o) = b[o]
    adjT16 = np.ascontiguousarray(np.asarray(adj, dtype=np.float32).T).astype(
        np.float16
    )
    return adjT16, mt16, bias512


def make_in_maps(x, adj, W, b):
    adjT16, mt16, bias512 = _host_prep(adj, W, b)
    x16 = np.asarray(x, dtype=np.float32).astype(np.float16)
    # [B, C, N, T] -> [B, 128 wp, 4 wt, T, C]
    xprop = np.ascontiguousarray(
        x16.reshape(B, C, NVT, P, T).transpose(0, 3, 2, 4, 1)
    )
    return [
        {
            "xprop": xprop[i],
            "adjT16": adjT16,
            "mt16": mt16,
            "bias512": bias512,
        }
        for i in range(B)
    ]


def _get_runner():
    """Reusable jitted SPMD executor (safe to invoke repeatedly, unlike
    per-call run_bass_kernel_spmd under axon)."""
    if "runner" in _NC_CACHE:
        return _NC_CACHE["runner"]
    import jax
    from jax.sharding import Mesh, PartitionSpec
    try:
        from jax import shard_map
    except ImportError:
        from jax.experimental.shard_map import shard_map
    from concourse import bass2jax, mybir

    nc = _get_nc()
    bass2jax.install_neuronx_cc_hook()

    pname = nc.partition_id_tensor.name if nc.partition_id_tensor else None
    in_names, out_names, out_avals, zero_outs = [], [], [], []
    for alloc in nc.m.functions[0].allocations:
        if not isinstance(alloc, mybir.MemoryLocationSet):
            continue
        name = alloc.memorylocations[0].name
        if alloc.kind == "ExternalInput":
            if name != pname:
                in_names.append(name)
        elif alloc.kind == "ExternalOutput":
            out_names.append(name)
            shape = tuple(alloc.tensor_shape)
            dtype = mybir.dt.np(alloc.dtype)
            out_avals.append(jax.core.ShapedArray(shape, dtype))
            zero_outs.append(np.zeros(shape, dtype))
    n_params = len(in_names)
    in_names_all = list(in_names) + out_names
    if pname is not None:
        in_names_all.append(pname)

    def _body(*args):
        operands = list(args)
        if pname is not None:
            operands.append(bass2jax.partition_id_tensor())
        return tuple(
            bass2jax._bass_exec_p.bind(
                *operands,
                out_avals=tuple(out_avals),
                in_names=tuple(in_names_all),
                out_names=tuple(out_names),
                lowering_input_output_aliases=(),
                sim_require_finite=True,
                sim_require_nnan=True,
                nc=nc,
            )
        )

    devices = jax.devices()[:B]
    mesh = Mesh(np.asarray(devices), ("core",))
    fn = jax.jit(
        shard_map(
            _body,
            mesh=mesh,
            in_specs=(PartitionSpec("core"),) * (n_params + len(out_names)),
            out_specs=(PartitionSpec("core"),) * len(out_names),
            check_rep=False,
        ),
        keep_unused=True,
    )

    def run(in_maps):
        per_core = [[np.asarray(m[nm]) for nm in in_names] for m in in_maps]
        concat_in = [
            np.concatenate([per_core[c][i] for c in range(B)], axis=0)
            for i in range(n_params)
        ]
        concat_zero = [np.concatenate([z] * B, axis=0) for z in zero_outs]
        outs = fn(*concat_in, *concat_zero)
        oi = out_names.index("out")
        full = np.asarray(outs[oi])
        per_core_rows = out_avals[oi].shape[0]
        return full.reshape(B, per_core_rows, *out_avals[oi].shape[1:])

    _NC_CACHE["runner"] = run
    return run


def _host_untranspose(outD):
    # device writes [NVT, P v, T, C o]; reference layout is [C, N, T]
    return np.ascontiguousarray(
        outD.reshape(B, N, T, C).transpose(0, 3, 1, 2)
    )


def kernel(x, adj, W, b):
    in_maps = make_in_maps(x, adj, W, b)
    try:
        run = _get_runner()
        return _host_untranspose(run(in_maps))
    except Exception:
        from concourse.bass_utils import run_bass_kernel_spmd

        res = run_bass_kernel_spmd(_get_nc(), in_maps, list(range(B)))
        outD = np.stack([res.results[i]["out"] for i in range(B)], axis=0)
        return _host_untranspose(outD)


# revision 18
# speedup vs baseline: 1.1648x; 1.1147x over previous
"""MixProp GNN message passing on 8 Trainium2 NeuronCores.

Reference computation (per batch element b):
    h0 = x;  h_k = alpha*x + (1-alpha) * (adj @ h_{k-1})   k=1..3   (matmul over nodes)
    ho = concat([h0..h3], channel axis);  out = W @ ho + b          (1x1 conv)

Node-propagation (node axis) commutes with channel mixing (channel
axis), so the alpha-blending folds into the conv weights on the host:
    out = sum_k M_k @ (A^k x) + b
with M_0 = W0 + a(W1+W2+W3), M_1 = B(W1 + aW2 + aW3),
     M_2 = B^2(W2 + aW3),    M_3 = B^3 W3,   (a=alpha, B=1-alpha)

Sharding: data-parallel over batch B=8, one batch element per core;
adj (host-pre-transposed) and conv weights replicated.

Device dataflow per core (fp16 operands, fp32 PSUM accumulation),
pipelined over T in chunks of 16 time steps:
  YK [128 w-part, 4 node tiles, 16 t, 128 kc]   kc = k*32 + c
    slot k=0 <- x (ACT copy from the contiguous chunk load)
    slot k   <- prop step k psum (PE contracts nodes; DVE/ACT evac)
  per node tile: XBAR DMA-transpose [v, (t,kc)] -> Z [kc, t, v] on-chip
    (no HBM scratch round trip)
  conv: one K=128 matmul per (node tile, t): psum[128 v, 32 o] with
    ap_size=32 (4x fewer PE rows than the [o, (v,t)] orientation)
  psum -> stage [v, o, t] f32 (DVE add folds the bias, (t,o)->(o,t))
  out DMA per node tile in two slabs: t 0:128 (512B descriptors,
  overlapped with the tail chunks) + t 128:168.
"""

import sys

import numpy as np

sys.path.insert(0, "/opt/trn_rl_repo")

from contextlib import ExitStack

GDEP = 3
ALPHA = 0.05
Y3_SCALE = 1.0 / 128.0   # keep |y3| inside fp16 range; folded into M3
C = 32            # channels
N = 512           # nodes
T = 168           # time steps
B = 8             # batch == n_cores
P = 128           # partitions
NVT = N // P      # 4 node tiles
KC = (GDEP + 1) * C   # 128 stacked (k, c) rows for the conv

TC = 24                                      # t-chunk size
CHUNKS = [(i * TC, TC) for i in range(7)]

_NC_CACHE = {}


def _build_nc():
    import concourse.mybir as mybir
    import concourse.tile as tile
    from concourse import bacc

    f32 = mybir.dt.float32
    f16 = mybir.dt.float16

    nc = bacc.Bacc("TRN2", target_bir_lowering=False, debug=False, num_devices=B)

    xprop = nc.dram_tensor("xprop", [P, NVT, T, C], f16, kind="ExternalInput").ap()
    adjT16 = nc.dram_tensor("adjT16", [N, N], f16, kind="ExternalInput").ap()
    mt16 = nc.dram_tensor("mt16", [KC, C], f16, kind="ExternalInput").ap()
    bias512 = nc.dram_tensor("bias512", [P, 512], f32, kind="ExternalInput").ap()
    out = nc.dram_tensor("out", [NVT, P, T, C], f32, kind="ExternalOutput").ap()

    with tile.TileContext(nc) as tc, ExitStack() as ctx:
        _emit(ctx, tc, nc, mybir, xprop, adjT16, mt16, bias512, out)

    nc.compile()
    return nc


def _emit(ctx, tc, nc, mybir, xprop, adjT16, mt16, bias512, out):
    f32 = mybir.dt.float32
    f16 = mybir.dt.float16
    Identity = mybir.ActivationFunctionType.Identity

    const_pool = ctx.enter_context(tc.tile_pool(name="const", bufs=1))
    xp_pool = ctx.enter_context(tc.tile_pool(name="xp", bufs=4))
    yk_pool = ctx.enter_context(tc.tile_pool(name="yk", bufs=4))
    z_pool = ctx.enter_context(tc.tile_pool(name="z", bufs=8))
    s_pool = ctx.enter_context(tc.tile_pool(name="s", bufs=4))
    psum_pool = ctx.enter_context(tc.tile_pool(name="psum", bufs=4, space="PSUM"))
    cpsum_pool = ctx.enter_context(tc.tile_pool(name="cpsum", bufs=4, space="PSUM"))

    # ---- startup: interleave adj / first x chunk per node tile so the
    # first step-1 matmul's dependencies land as early as possible ----
    adj_sb = const_pool.tile([P, NVT, N], f16, tag="adj")
    adj_v = adjT16.rearrange("(wt wp) v -> wp wt v", wp=P)
    xp0 = xp_pool.tile([P, NVT, TC, C], f16, tag="xp")
    t00, tn0 = CHUNKS[0]
    nc.sync.dma_start(adj_sb[:, 0], adj_v[:, 0])
    nc.sync.dma_start(xp0[:, 0, :tn0, :], xprop[:, 0, t00:t00 + tn0, :])
    for wt in range(1, NVT):
        nc.sync.dma_start(adj_sb[:, wt], adj_v[:, wt])
        nc.sync.dma_start(xp0[:, wt, :tn0, :], xprop[:, wt, t00:t00 + tn0, :])

    mt_sb = const_pool.tile([KC, C], f16, tag="mt")
    bias_sb = const_pool.tile([P, 512], f32, tag="bias")
    nc.sync.dma_start(mt_sb[:], mt16)
    nc.sync.dma_start(bias_sb[:], bias512)
    xp1 = xp_pool.tile([P, NVT, TC, C], f16, tag="xp")
    t01, tn1 = CHUNKS[1]
    nc.sync.dma_start(xp1[:, :, :tn1, :], xprop[:, :, t01:t01 + tn1, :])
    xp2 = xp_pool.tile([P, NVT, TC, C], f16, tag="xp")
    t02, tn2 = CHUNKS[2]
    nc.sync.dma_start(xp2[:, :, :tn2, :], xprop[:, :, t02:t02 + tn2, :])

    def emit_steps(ci, t0, tn, xp, after_step):
        """3 propagation steps + per-node-tile XBAR transpose; invokes
        after_step(s) between steps to fill PE stall slots with the
        previous chunk's conv."""
        yk = yk_pool.tile([P, NVT, TC, KC], f16, tag="yk")
        zs = []
        for k in (1, 2, 3):
            for vt in range(NVT):
                for ta in range(0, tn, 16):
                    tb = min(ta + 16, tn)
                    ps = psum_pool.tile([P, 512], f32, tag="ps")
                    for wt in range(NVT):
                        rhs = (xp[:, wt, ta:tb, :] if k == 1
                               else yk[:, wt, ta:tb, C * (k - 1):C * k])
                        nc.tensor.matmul(
                            ps[:, :(tb - ta) * C],
                            adj_sb[:, wt, vt * P:(vt + 1) * P],
                            rhs,
                            start=(wt == 0),
                            stop=(wt == NVT - 1),
                        )
                    src = ps[:, :(tb - ta) * C].rearrange(
                        "p (t c) -> p t c", c=C
                    )
                    dst = yk[:, vt, ta:tb, C * k:C * (k + 1)]
                    if k == 3 or (vt < 2):
                        nc.vector.tensor_copy(dst, src) if k < 3 else                             nc.scalar.activation(dst, src, Identity,
                                                 scale=Y3_SCALE)
                    else:
                        nc.scalar.activation(dst, src, Identity)
                if k == 3:
                    # x -> stacked slot k=0, per node tile, just before
                    # its XBAR (keeps the ACT queue free of long waits)
                    nc.scalar.activation(
                        yk[:, vt, :tn, 0:C], xp[:, vt, :tn, :], Identity
                    )
                    z = z_pool.tile([P, TC, P], f16, tag="z")
                    nc.sync.dma_start(
                        z[:, :tn, :], yk[:, vt, :tn, :], transpose=True
                    )
                    zs.append(z)
            after_step(k)
        return (t0, tn, zs)

    def emit_conv_vt(state, vt):
        """Conv for one node tile of a finished chunk: tn ap32 matmuls
        into one psum bank, bias-add evac (contiguous (t,o)), flush."""
        t0, tn, zs = state
        z = zs[vt]
        s = s_pool.tile([P, TC * C], f32, tag="s")
        for ta in range(0, tn, 16):
            tb = min(ta + 16, tn)
            cps = cpsum_pool.tile([P, 512], f32, tag="cps")
            for i in range(ta, tb):
                nc.tensor.matmul(
                    cps[:, C * (i - ta):C * (i - ta + 1)],
                    z[:, i, :],
                    mt_sb[:],
                    start=True,
                    stop=True,
                )
            nc.vector.tensor_add(
                s[:, ta * C:tb * C],
                cps[:, :(tb - ta) * C],
                bias_sb[:, :(tb - ta) * C],
            )
        nc.sync.dma_start(
            out[vt, :, t0:t0 + tn, :],
            s[:, :tn * C].rearrange("p (t o) -> p t o", o=C),
        )

    pend = []                      # chunks whose conv is not yet emitted
    xps = {0: xp0, 1: xp1, 2: xp2}
    for ci, (t0, tn) in enumerate(CHUNKS):
        # prefetch x two chunks ahead: the SP queue head-of-line
        # blocks on the previous chunk's last XBAR semaphore, so a
        # distance-1 prefetch only lands at the chunk boundary; at
        # distance 2 the load is in flight a full chunk before use
        if ci + 3 < len(CHUNKS):
            nt0, ntn = CHUNKS[ci + 3]
            nxp = xp_pool.tile([P, NVT, TC, C], f16, tag="xp")
            nc.sync.dma_start(nxp[:, :, :ntn, :], xprop[:, :, nt0:nt0 + ntn, :])
            xps[ci + 3] = nxp
        xp = xps.pop(ci)

        # conv lagged two chunks behind the propagation: the
        # evac3 -> XBAR -> conv semaphore chain is ~7us; a full chunk
        # of step matmuls (~11us) must sit between step3(c) and
        # conv(c) or the PE stalls (and drops out of max p-state)
        prev = pend.pop(0) if len(pend) >= 2 else None

        def after_step(k, prev=prev):
            if prev is None:
                return
            if k == 1:
                emit_conv_vt(prev, 0)
                emit_conv_vt(prev, 1)
            elif k == 2:
                emit_conv_vt(prev, 2)
                emit_conv_vt(prev, 3)

        pend.append(emit_steps(ci, t0, tn, xp, after_step))
    for state in pend:
        for vt in range(NVT):
            emit_conv_vt(state, vt)


def _get_nc():
    if "nc" not in _NC_CACHE:
        _NC_CACHE["nc"] = _build_nc()
    return _NC_CACHE["nc"]


def _host_prep(adj, W, b):
    """Host-side constant folding: transposed adj, mixed conv weights."""
    a, beta = ALPHA, 1.0 - ALPHA
    W = np.asarray(W, dtype=np.float32)
    W0, W1, W2, W3 = (W[:, i * C:(i + 1) * C] for i in range(4))
    M0 = W0 + a * (W1 + W2 + W3)
    M1 = beta * (W1 + a * W2 + a * W3)
    M2 = beta * beta * (W2 + a * W3)
    M3 = beta * beta * beta * W3 / Y3_SCALE
    mt16 = np.ascontiguousarray(
        np.concatenate([M0.T, M1.T, M2.T, M3.T], axis=0)
    ).astype(np.float16)  # [128, 32]: row (k*32+c), col o = M_k[o, c]
    bias512 = np.ascontiguousarray(
        np.tile(np.asarray(b, dtype=np.float32)[None, :], (P, TC))
    )  # [128, 512]: col (t'*32+o) = b[o]
    adjT16 = np.ascontiguousarray(np.asarray(adj, dtype=np.float32).T).astype(
        np.float16
    )
    return adjT16, mt16, bias512


def make_in_maps(x, adj, W, b):
    adjT16, mt16, bias512 = _host_prep(adj, W, b)
    x16 = np.asarray(x, dtype=np.float32).astype(np.float16)
    # [B, C, N, T] -> [B, 128 wp, 4 wt, T, C]
    xprop = np.ascontiguousarray(
        x16.reshape(B, C, NVT, P, T).transpose(0, 3, 2, 4, 1)
    )
    return [
        {
            "xprop": xprop[i],
            "adjT16": adjT16,
            "mt16": mt16,
            "bias512": bias512,
        }
        for i in range(B)
    ]


def _get_runner():
    """Reusable jitted SPMD executor (safe to invoke repeatedly, unlike
    per-call run_bass_kernel_spmd under axon)."""
    if "runner" in _NC_CACHE:
        return _NC_CACHE["runner"]
    import jax
    from jax.sharding import Mesh, PartitionSpec
    try:
        from jax import shard_map
    except ImportError:
        from jax.experimental.shard_map import shard_map
    from concourse import bass2jax, mybir

    nc = _get_nc()
    bass2jax.install_neuronx_cc_hook()

    pname = nc.partition_id_tensor.name if nc.partition_id_tensor else None
    in_names, out_names, out_avals, zero_outs = [], [], [], []
    for alloc in nc.m.functions[0].allocations:
        if not isinstance(alloc, mybir.MemoryLocationSet):
            continue
        name = alloc.memorylocations[0].name
        if alloc.kind == "ExternalInput":
            if name != pname:
                in_names.append(name)
        elif alloc.kind == "ExternalOutput":
            out_names.append(name)
            shape = tuple(alloc.tensor_shape)
            dtype = mybir.dt.np(alloc.dtype)
            out_avals.append(jax.core.ShapedArray(shape, dtype))
            zero_outs.append(np.zeros(shape, dtype))
    n_params = len(in_names)
    in_names_all = list(in_names) + out_names
    if pname is not None:
        in_names_all.append(pname)

    def _body(*args):
        operands = list(args)
        if pname is not None:
            operands.append(bass2jax.partition_id_tensor())
        return tuple(
            bass2jax._bass_exec_p.bind(
                *operands,
                out_avals=tuple(out_avals),
                in_names=tuple(in_names_all),
                out_names=tuple(out_names),
                lowering_input_output_aliases=(),
                sim_require_finite=True,
                sim_require_nnan=True,
                nc=nc,
            )
        )

    devices = jax.devices()[:B]
    mesh = Mesh(np.asarray(devices), ("core",))
    fn = jax.jit(
        shard_map(
            _body,
            mesh=mesh,
            in_specs=(PartitionSpec("core"),) * (n_params + len(out_names)),
            out_specs=(PartitionSpec("core"),) * len(out_names),
            check_rep=False,
        ),
        keep_unused=True,
    )

    def run(in_maps):
        per_core = [[np.asarray(m[nm]) for nm in in_names] for m in in_maps]
        concat_in = [
            np.concatenate([per_core[c][i] for c in range(B)], axis=0)
            for i in range(n_params)
        ]
        concat_zero = [np.concatenate([z] * B, axis=0) for z in zero_outs]
        outs = fn(*concat_in, *concat_zero)
        oi = out_names.index("out")
        full = np.asarray(outs[oi])
        per_core_rows = out_avals[oi].shape[0]
        return full.reshape(B, per_core_rows, *out_avals[oi].shape[1:])

    _NC_CACHE["runner"] = run
    return run


def _host_untranspose(outD):
    # device writes [NVT, P v, T, C o]; reference layout is [C, N, T]
    return np.ascontiguousarray(
        outD.reshape(B, N, T, C).transpose(0, 3, 1, 2)
    )


def kernel(x, adj, W, b):
    in_maps = make_in_maps(x, adj, W, b)
    try:
        run = _get_runner()
        return _host_untranspose(run(in_maps))
    except Exception:
        from concourse.bass_utils import run_bass_kernel_spmd

        res = run_bass_kernel_spmd(_get_nc(), in_maps, list(range(B)))
        outD = np.stack([res.results[i]["out"] for i in range(B)], axis=0)
        return _host_untranspose(outD)


# revision 19
# speedup vs baseline: 1.2020x; 1.0319x over previous
"""MixProp GNN message passing on 8 Trainium2 NeuronCores.

Reference computation (per batch element b):
    h0 = x;  h_k = alpha*x + (1-alpha) * (adj @ h_{k-1})   k=1..3   (matmul over nodes)
    ho = concat([h0..h3], channel axis);  out = W @ ho + b          (1x1 conv)

Node-propagation (node axis) commutes with channel mixing (channel
axis), so the alpha-blending folds into the conv weights on the host:
    out = sum_k M_k @ (A^k x) + b
with M_0 = W0 + a(W1+W2+W3), M_1 = B(W1 + aW2 + aW3),
     M_2 = B^2(W2 + aW3),    M_3 = B^3 W3,   (a=alpha, B=1-alpha)

Sharding: data-parallel over batch B=8, one batch element per core;
adj (host-pre-transposed) and conv weights replicated.

Device dataflow per core (fp16 operands, fp32 PSUM accumulation),
pipelined over T in chunks of 16 time steps:
  YK [128 w-part, 4 node tiles, 16 t, 128 kc]   kc = k*32 + c
    slot k=0 <- x (ACT copy from the contiguous chunk load)
    slot k   <- prop step k psum (PE contracts nodes; DVE/ACT evac)
  per node tile: XBAR DMA-transpose [v, (t,kc)] -> Z [kc, t, v] on-chip
    (no HBM scratch round trip)
  conv: one K=128 matmul per (node tile, t): psum[128 v, 32 o] with
    ap_size=32 (4x fewer PE rows than the [o, (v,t)] orientation)
  psum -> stage [v, o, t] f32 (DVE add folds the bias, (t,o)->(o,t))
  out DMA per node tile in two slabs: t 0:128 (512B descriptors,
  overlapped with the tail chunks) + t 128:168.
"""

import sys

import numpy as np

sys.path.insert(0, "/opt/trn_rl_repo")

from contextlib import ExitStack

GDEP = 3
ALPHA = 0.05
Y3_SCALE = 1.0 / 128.0   # keep |y3| inside fp16 range; folded into M3
C = 32            # channels
N = 512           # nodes
T = 168           # time steps
B = 8             # batch == n_cores
P = 128           # partitions
NVT = N // P      # 4 node tiles
KC = (GDEP + 1) * C   # 128 stacked (k, c) rows for the conv

TC = 28                                      # t-chunk size
CHUNKS = [(i * TC, TC) for i in range(6)]

_NC_CACHE = {}


def _build_nc():
    import concourse.mybir as mybir
    import concourse.tile as tile
    from concourse import bacc

    f32 = mybir.dt.float32
    f16 = mybir.dt.float16

    nc = bacc.Bacc("TRN2", target_bir_lowering=False, debug=False, num_devices=B)

    xprop = nc.dram_tensor("xprop", [P, NVT, T, C], f16, kind="ExternalInput").ap()
    adjT16 = nc.dram_tensor("adjT16", [N, N], f16, kind="ExternalInput").ap()
    mt16 = nc.dram_tensor("mt16", [KC, C], f16, kind="ExternalInput").ap()
    bias512 = nc.dram_tensor("bias512", [P, 512], f32, kind="ExternalInput").ap()
    out = nc.dram_tensor("out", [NVT, P, T, C], f32, kind="ExternalOutput").ap()

    with tile.TileContext(nc) as tc, ExitStack() as ctx:
        _emit(ctx, tc, nc, mybir, xprop, adjT16, mt16, bias512, out)

    nc.compile()
    return nc


def _emit(ctx, tc, nc, mybir, xprop, adjT16, mt16, bias512, out):
    f32 = mybir.dt.float32
    f16 = mybir.dt.float16
    Identity = mybir.ActivationFunctionType.Identity

    const_pool = ctx.enter_context(tc.tile_pool(name="const", bufs=1))
    xp_pool = ctx.enter_context(tc.tile_pool(name="xp", bufs=4))
    yk_pool = ctx.enter_context(tc.tile_pool(name="yk", bufs=3))
    z_pool = ctx.enter_context(tc.tile_pool(name="z", bufs=8))
    s_pool = ctx.enter_context(tc.tile_pool(name="s", bufs=4))
    psum_pool = ctx.enter_context(tc.tile_pool(name="psum", bufs=4, space="PSUM"))
    cpsum_pool = ctx.enter_context(tc.tile_pool(name="cpsum", bufs=4, space="PSUM"))

    # ---- startup: interleave adj / first x chunk per node tile so the
    # first step-1 matmul's dependencies land as early as possible ----
    adj_sb = const_pool.tile([P, NVT, N], f16, tag="adj")
    adj_v = adjT16.rearrange("(wt wp) v -> wp wt v", wp=P)
    xp0 = xp_pool.tile([P, NVT, TC, C], f16, tag="xp")
    t00, tn0 = CHUNKS[0]
    nc.sync.dma_start(adj_sb[:, 0], adj_v[:, 0])
    nc.sync.dma_start(xp0[:, 0, :tn0, :], xprop[:, 0, t00:t00 + tn0, :])
    for wt in range(1, NVT):
        nc.sync.dma_start(adj_sb[:, wt], adj_v[:, wt])
        nc.sync.dma_start(xp0[:, wt, :tn0, :], xprop[:, wt, t00:t00 + tn0, :])

    mt_sb = const_pool.tile([KC, C], f16, tag="mt")
    bias_sb = const_pool.tile([P, 512], f32, tag="bias")
    nc.sync.dma_start(mt_sb[:], mt16)
    nc.sync.dma_start(bias_sb[:], bias512)
    xp1 = xp_pool.tile([P, NVT, TC, C], f16, tag="xp")
    t01, tn1 = CHUNKS[1]
    nc.sync.dma_start(xp1[:, :, :tn1, :], xprop[:, :, t01:t01 + tn1, :])
    xp2 = xp_pool.tile([P, NVT, TC, C], f16, tag="xp")
    t02, tn2 = CHUNKS[2]
    nc.sync.dma_start(xp2[:, :, :tn2, :], xprop[:, :, t02:t02 + tn2, :])

    def emit_steps(ci, t0, tn, xp, after_step):
        """3 propagation steps + per-node-tile XBAR transpose; invokes
        after_step(s) between steps to fill PE stall slots with the
        previous chunk's conv."""
        yk = yk_pool.tile([P, NVT, TC, KC], f16, tag="yk")
        zs = []
        for k in (1, 2, 3):
            for vt in range(NVT):
                for ta in range(0, tn, 16):
                    tb = min(ta + 16, tn)
                    ps = psum_pool.tile([P, 512], f32, tag="ps")
                    for wt in range(NVT):
                        rhs = (xp[:, wt, ta:tb, :] if k == 1
                               else yk[:, wt, ta:tb, C * (k - 1):C * k])
                        nc.tensor.matmul(
                            ps[:, :(tb - ta) * C],
                            adj_sb[:, wt, vt * P:(vt + 1) * P],
                            rhs,
                            start=(wt == 0),
                            stop=(wt == NVT - 1),
                        )
                    src = ps[:, :(tb - ta) * C].rearrange(
                        "p (t c) -> p t c", c=C
                    )
                    dst = yk[:, vt, ta:tb, C * k:C * (k + 1)]
                    if k == 3 or (vt < 2):
                        nc.vector.tensor_copy(dst, src) if k < 3 else                             nc.scalar.activation(dst, src, Identity,
                                                 scale=Y3_SCALE)
                    else:
                        nc.scalar.activation(dst, src, Identity)
                if k == 3:
                    # x -> stacked slot k=0, per node tile, just before
                    # its XBAR (keeps the ACT queue free of long waits)
                    nc.scalar.activation(
                        yk[:, vt, :tn, 0:C], xp[:, vt, :tn, :], Identity
                    )
                    z = z_pool.tile([P, TC, P], f16, tag="z")
                    nc.sync.dma_start(
                        z[:, :tn, :], yk[:, vt, :tn, :], transpose=True
                    )
                    zs.append(z)
            after_step(k)
        return (t0, tn, zs)

    def emit_conv_vt(state, vt):
        """Conv for one node tile of a finished chunk: tn ap32 matmuls
        into one psum bank, bias-add evac (contiguous (t,o)), flush."""
        t0, tn, zs = state
        z = zs[vt]
        s = s_pool.tile([P, TC * C], f32, tag="s")
        for ta in range(0, tn, 16):
            tb = min(ta + 16, tn)
            cps = cpsum_pool.tile([P, 512], f32, tag="cps")
            for i in range(ta, tb):
                nc.tensor.matmul(
                    cps[:, C * (i - ta):C * (i - ta + 1)],
                    z[:, i, :],
                    mt_sb[:],
                    start=True,
                    stop=True,
                )
            nc.vector.tensor_add(
                s[:, ta * C:tb * C],
                cps[:, :(tb - ta) * C],
                bias_sb[:, :(tb - ta) * C],
            )
        nc.sync.dma_start(
            out[vt, :, t0:t0 + tn, :],
            s[:, :tn * C].rearrange("p (t o) -> p t o", o=C),
        )

    pend = []                      # chunks whose conv is not yet emitted
    xps = {0: xp0, 1: xp1, 2: xp2}
    for ci, (t0, tn) in enumerate(CHUNKS):
        # prefetch x two chunks ahead: the SP queue head-of-line
        # blocks on the previous chunk's last XBAR semaphore, so a
        # distance-1 prefetch only lands at the chunk boundary; at
        # distance 2 the load is in flight a full chunk before use
        if ci + 3 < len(CHUNKS):
            nt0, ntn = CHUNKS[ci + 3]
            nxp = xp_pool.tile([P, NVT, TC, C], f16, tag="xp")
            nc.sync.dma_start(nxp[:, :, :ntn, :], xprop[:, :, nt0:nt0 + ntn, :])
            xps[ci + 3] = nxp
        xp = xps.pop(ci)

        # conv lagged two chunks behind the propagation: the
        # evac3 -> XBAR -> conv semaphore chain is ~7us; a full chunk
        # of step matmuls (~11us) must sit between step3(c) and
        # conv(c) or the PE stalls (and drops out of max p-state)
        prev = pend.pop(0) if len(pend) >= 2 else None

        def after_step(k, prev=prev):
            if prev is None:
                return
            if k == 1:
                emit_conv_vt(prev, 0)
                emit_conv_vt(prev, 1)
            elif k == 2:
                emit_conv_vt(prev, 2)
                emit_conv_vt(prev, 3)

        pend.append(emit_steps(ci, t0, tn, xp, after_step))
    for state in pend:
        for vt in range(NVT):
            emit_conv_vt(state, vt)


def _get_nc():
    if "nc" not in _NC_CACHE:
        _NC_CACHE["nc"] = _build_nc()
    return _NC_CACHE["nc"]


def _host_prep(adj, W, b):
    """Host-side constant folding: transposed adj, mixed conv weights."""
    a, beta = ALPHA, 1.0 - ALPHA
    W = np.asarray(W, dtype=np.float32)
    W0, W1, W2, W3 = (W[:, i * C:(i + 1) * C] for i in range(4))
    M0 = W0 + a * (W1 + W2 + W3)
    M1 = beta * (W1 + a * W2 + a * W3)
    M2 = beta * beta * (W2 + a * W3)
    M3 = beta * beta * beta * W3 / Y3_SCALE
    mt16 = np.ascontiguousarray(
        np.concatenate([M0.T, M1.T, M2.T, M3.T], axis=0)
    ).astype(np.float16)  # [128, 32]: row (k*32+c), col o = M_k[o, c]
    bias512 = np.ascontiguousarray(
        np.tile(np.asarray(b, dtype=np.float32)[None, :], (P, TC))
    )  # [128, 512]: col (t'*32+o) = b[o]
    adjT16 = np.ascontiguousarray(np.asarray(adj, dtype=np.float32).T).astype(
        np.float16
    )
    return adjT16, mt16, bias512


def make_in_maps(x, adj, W, b):
    adjT16, mt16, bias512 = _host_prep(adj, W, b)
    x16 = np.asarray(x, dtype=np.float32).astype(np.float16)
    # [B, C, N, T] -> [B, 128 wp, 4 wt, T, C]
    xprop = np.ascontiguousarray(
        x16.reshape(B, C, NVT, P, T).transpose(0, 3, 2, 4, 1)
    )
    return [
        {
            "xprop": xprop[i],
            "adjT16": adjT16,
            "mt16": mt16,
            "bias512": bias512,
        }
        for i in range(B)
    ]


def _get_runner():
    """Reusable jitted SPMD executor (safe to invoke repeatedly, unlike
    per-call run_bass_kernel_spmd under axon)."""
    if "runner" in _NC_CACHE:
        return _NC_CACHE["runner"]
    import jax
    from jax.sharding import Mesh, PartitionSpec
    try:
        from jax import shard_map
    except ImportError:
        from jax.experimental.shard_map import shard_map
    from concourse import bass2jax, mybir

    nc = _get_nc()
    bass2jax.install_neuronx_cc_hook()

    pname = nc.partition_id_tensor.name if nc.partition_id_tensor else None
    in_names, out_names, out_avals, zero_outs = [], [], [], []
    for alloc in nc.m.functions[0].allocations:
        if not isinstance(alloc, mybir.MemoryLocationSet):
            continue
        name = alloc.memorylocations[0].name
        if alloc.kind == "ExternalInput":
            if name != pname:
                in_names.append(name)
        elif alloc.kind == "ExternalOutput":
            out_names.append(name)
            shape = tuple(alloc.tensor_shape)
            dtype = mybir.dt.np(alloc.dtype)
            out_avals.append(jax.core.ShapedArray(shape, dtype))
            zero_outs.append(np.zeros(shape, dtype))
    n_params = len(in_names)
    in_names_all = list(in_names) + out_names
    if pname is not None:
        in_names_all.append(pname)

    def _body(*args):
        operands = list(args)
        if pname is not None:
            operands.append(bass2jax.partition_id_tensor())
        return tuple(
            bass2jax._bass_exec_p.bind(
                *operands,
                out_avals=tuple(out_avals),
                in_names=tuple(in_names_all),
                out_names=tuple(out_names),
                lowering_input_output_aliases=(),
                sim_require_finite=True,
                sim_require_nnan=True,
                nc=nc,
            )
        )

    devices = jax.devices()[:B]
    mesh = Mesh(np.asarray(devices), ("core",))
    fn = jax.jit(
        shard_map(
            _body,
            mesh=mesh,
            in_specs=(PartitionSpec("core"),) * (n_params + len(out_names)),
            out_specs=(PartitionSpec("core"),) * len(out_names),
            check_rep=False,
        ),
        keep_unused=True,
    )

    def run(in_maps):
        per_core = [[np.asarray(m[nm]) for nm in in_names] for m in in_maps]
        concat_in = [
            np.concatenate([per_core[c][i] for c in range(B)], axis=0)
            for i in range(n_params)
        ]
        concat_zero = [np.concatenate([z] * B, axis=0) for z in zero_outs]
        outs = fn(*concat_in, *concat_zero)
        oi = out_names.index("out")
        full = np.asarray(outs[oi])
        per_core_rows = out_avals[oi].shape[0]
        return full.reshape(B, per_core_rows, *out_avals[oi].shape[1:])

    _NC_CACHE["runner"] = run
    return run


def _host_untranspose(outD):
    # device writes [NVT, P v, T, C o]; reference layout is [C, N, T]
    return np.ascontiguousarray(
        outD.reshape(B, N, T, C).transpose(0, 3, 1, 2)
    )


def kernel(x, adj, W, b):
    in_maps = make_in_maps(x, adj, W, b)
    try:
        run = _get_runner()
        return _host_untranspose(run(in_maps))
    except Exception:
        from concourse.bass_utils import run_bass_kernel_spmd

        res = run_bass_kernel_spmd(_get_nc(), in_maps, list(range(B)))
        outD = np.stack([res.results[i]["out"] for i in range(B)], axis=0)
        return _host_untranspose(outD)
